# revision 9
# baseline (speedup 1.0000x reference)
"""DCN cross-layer kernel for Trainium2 (8 NeuronCores, data-parallel).

Reference computation (L=3 layers):
    x_{l+1} = x0 * (x_l . w_l) + b_l + x_l

Algebraic collapse used here: writing x_l = x0 * sigma_l + B_l, where
sigma_l is a per-row scalar and B_l = sum_{j<l} b_j is a broadcast
vector, the recurrence becomes
    d_l      = x0 . w_l                  (per-row dot, original x0!)
    sigma_{l+1} = sigma_l * (1 + d_l) + beta_l,   beta_l = B_l . w_l (host const)
    out      = x0 * sigma_3 + B_3
So the device kernel is one streaming pass over x.  Memory-bound:
8 MiB HBM traffic per core (load x once, store out once).

Engine split (per [128, 1024] tile):
  PE   : 8 transposes (128x128) + 8 dot matmuls (K=128, M=128, N=3)
         accumulating d = x0T . W^T into PSUM [128 batch, 3]
  DVE  : 2 PSUM->SBUF copies of the transposed tile + 3 tiny [128,1]
         recurrence ops
  ACT  : final scaled copy out = x0 * sigma_3
  DMA  : 512 KiB load + 512 KiB store
"""

import numpy as np

N_CORES = 8
B, D = 8192, 1024
L = 3
B_SH = B // N_CORES  # 1024 rows per core
P = 128
N_TILES = B_SH // P  # 8 tiles of [128, 1024] per core
N_CH = D // P        # 8 d-chunks per tile

LAST_RESULTS = None  # BassKernelResults of the most recent run (for test.py)


def _build_program(betas, has_b3):
    import concourse.bacc as bacc
    import concourse.tile as tile
    from concourse import mybir
    from concourse.masks import make_identity

    f32 = mybir.dt.float32
    mult = mybir.AluOpType.mult
    add = mybir.AluOpType.add

    nc = bacc.Bacc("TRN2", target_bir_lowering=False, debug=False,
                   num_devices=N_CORES)

    x_d = nc.dram_tensor("x", [B_SH, D], f32, kind="ExternalInput").ap()
    wt_d = nc.dram_tensor("wt", [D, L], f32, kind="ExternalInput").ap()
    out_d = nc.dram_tensor("out", [B_SH, D], f32, kind="ExternalOutput").ap()
    b3_d = None
    if has_b3:
        b3_d = nc.dram_tensor("b3b", [P, D], f32, kind="ExternalInput").ap()

    with tile.TileContext(nc) as tc:
        with (
            tc.tile_pool(name="const", bufs=1) as const_pool,
            tc.tile_pool(name="xin", bufs=3) as xin,
            tc.tile_pool(name="xtp", bufs=3) as xtp,
            tc.tile_pool(name="outp", bufs=3) as outp,
            tc.tile_pool(name="small", bufs=3) as small,
            tc.tile_pool(name="ptp", bufs=4, space="PSUM") as ptp,
            tc.tile_pool(name="pd", bufs=3, space="PSUM") as pd,
        ):
            ident = const_pool.tile([P, P], f32, tag="ident")
            make_identity(nc, ident[:])
            # W^T chunks: wt_sb[p, c, l] = W[l, c*128+p]
            wt_sb = const_pool.tile([P, N_CH, L], f32, tag="wt")
            nc.sync.dma_start(wt_sb[:],
                              wt_d.rearrange("(c p) l -> p c l", p=P))
            if has_b3:
                b3 = const_pool.tile([P, D], f32, tag="b3")
                nc.sync.dma_start(b3[:], b3_d[:])

            for t in range(N_TILES):
                xt = xin.tile([P, D], f32, tag="xt")
                nc.sync.dma_start(xt[:], x_d[t * P:(t + 1) * P, :])

                # transpose the tile: xts[p, c*128+a] = xt[a, c*128+p]
                xts = xtp.tile([P, D], f32, tag="xts")
                for g in range(N_CH // 4):
                    tp = ptp.tile([P, 512], f32, tag="tp")
                    for j in range(4):
                        c = 4 * g + j
                        nc.tensor.transpose(
                            tp[:, j * P:(j + 1) * P],
                            xt[:, c * P:(c + 1) * P],
                            ident[:])
                    nc.vector.tensor_copy(xts[:, g * 512:(g + 1) * 512],
                                          tp[:])

                # d[b, l] = sum_d x0[b, d] W[l, d] via PE, accumulated
                # over the 8 d-chunks
                dps = pd.tile([P, L], f32, tag="dps")
                for c in range(N_CH):
                    nc.tensor.matmul(
                        dps[:],
                        xts[:, c * P:(c + 1) * P],
                        wt_sb[:, c, :],
                        start=(c == 0),
                        stop=(c == N_CH - 1))

                # sigma recurrence: sigma_{l+1} = sigma_l*(1+d_l) + beta_l
                sig = small.tile([P, 1], f32, tag="sig1")
                nc.vector.tensor_scalar_add(sig[:], dps[:, 0:1],
                                            1.0 + betas[0])
                for l in (1, 2):
                    nsig = small.tile([P, 1], f32, tag=f"sig{l + 1}")
                    nc.vector.scalar_tensor_tensor(
                        out=nsig[:], in0=dps[:, l:l + 1], scalar=1.0,
                        in1=sig[:], op0=add, op1=mult)
                    if betas[l] != 0.0:
                        nc.vector.tensor_scalar_add(nsig[:], nsig[:],
                                                    float(betas[l]))
                    sig = nsig

                # out = x0 * sigma_3 (+ B3) — scaled copy on ACT
                ot = outp.tile([P, D], f32, tag="ot")
                nc.scalar.mul(ot[:], xt[:], sig[:])
                if has_b3:
                    nc.vector.tensor_add(ot[:], ot[:], b3[:])
                nc.sync.dma_start(out_d[t * P:(t + 1) * P, :], ot[:])

    nc.compile()
    return nc


def predict_time_ns(trace_path=None):
    """Single-core timeline-sim of the kernel program (cost-model time in
    ns).  SPMD data-parallel with no collectives, so per-core time ==
    kernel time.  Optionally writes a perfetto trace."""
    from trails.perfetto import LazyPerfetto
    for _m in ("enable_explicit_ordering", "reserve_process_order",
               "add_counter", "add_flow", "add_instant"):
        if not hasattr(LazyPerfetto, _m):
            setattr(LazyPerfetto, _m, lambda self, *a, **k: None)
    from concourse.timeline_sim import TimelineSim

    nc = _build_program([0.0, 0.0, 0.0], False)
    tlsim = TimelineSim(nc, trace=trace_path is not None)
    tlsim.simulate()
    if trace_path is not None and tlsim.perfetto is not None:
        tlsim.perfetto.save(trace_path)
    return tlsim.time


def kernel(x, W, b):
    global LAST_RESULTS
    from concourse.bass_utils import run_bass_kernel_spmd

    x = np.ascontiguousarray(np.asarray(x, dtype=np.float32))
    W = np.asarray(W, dtype=np.float32)
    b = np.asarray(b, dtype=np.float32)

    # Host precompute: beta_l = (sum_{j<l} b_j) . w_l  and B_3 = sum_l b_l.
    Bl = np.zeros(D, dtype=np.float64)
    betas = []
    for l in range(L):
        betas.append(float(Bl @ W[l].astype(np.float64)))
        Bl = Bl + b[l].astype(np.float64)
    B3 = Bl.astype(np.float32)
    has_b3 = bool(np.any(B3))

    nc = _build_program(betas, has_b3)

    wt_host = np.ascontiguousarray(W.T)  # [D, L]
    in_maps = []
    for i in range(N_CORES):
        m = {"x": x[i * B_SH:(i + 1) * B_SH], "wt": wt_host}
        if has_b3:
            m["b3b"] = np.ascontiguousarray(np.broadcast_to(B3, (P, D)))
        in_maps.append(m)

    res = run_bass_kernel_spmd(nc, in_maps, core_ids=list(range(N_CORES)))
    LAST_RESULTS = res
    out = np.concatenate([res.results[i]["out"] for i in range(N_CORES)],
                         axis=0)
    return out


# revision 10
# speedup vs baseline: 1.3920x; 1.3920x over previous
"""DCN cross-layer kernel for Trainium2 (8 NeuronCores, data-parallel).

Reference computation (L=3 layers):
    x_{l+1} = x0 * (x_l . w_l) + b_l + x_l

Algebraic collapse used here: writing x_l = x0 * sigma_l + B_l, where
sigma_l is a per-row scalar and B_l = sum_{j<l} b_j is a broadcast
vector, the recurrence becomes
    d_l      = x0 . w_l                  (per-row dot, original x0!)
    sigma_{l+1} = sigma_l * (1 + d_l) + beta_l,   beta_l = B_l . w_l (host const)
    out      = x0 * sigma_3 + B_3
So the device kernel is one streaming pass over x.  Memory-bound:
8 MiB HBM traffic per core (load x once, store out once).

Engine split (per [128, 1024] tile):
  PE   : 8 transposes (128x128) + 8 dot matmuls (K=128, M=128, N=3)
         accumulating d = x0T . W^T into PSUM [128 batch, 3]
  DVE  : 2 PSUM->SBUF copies of the transposed tile + 3 tiny [128,1]
         recurrence ops
  ACT  : final scaled copy out = x0 * sigma_3
  DMA  : 512 KiB load + 512 KiB store
"""

import numpy as np

N_CORES = 8
B, D = 8192, 1024
L = 3
B_SH = B // N_CORES  # 1024 rows per core
P = 128
N_TILES = B_SH // P  # 8 tiles of [128, 1024] per core
N_CH = D // P        # 8 d-chunks per tile

LAST_RESULTS = None  # BassKernelResults of the most recent run (for test.py)


def _build_program(betas, has_b3):
    import concourse.bacc as bacc
    import concourse.tile as tile
    from concourse import mybir
    from concourse.masks import make_identity

    f32 = mybir.dt.float32
    mult = mybir.AluOpType.mult
    add = mybir.AluOpType.add

    nc = bacc.Bacc("TRN2", target_bir_lowering=False, debug=False,
                   num_devices=N_CORES)

    x_d = nc.dram_tensor("x", [B_SH, D], f32, kind="ExternalInput").ap()
    wt_d = nc.dram_tensor("wt", [D, L], f32, kind="ExternalInput").ap()
    out_d = nc.dram_tensor("out", [B_SH, D], f32, kind="ExternalOutput").ap()
    b3_d = None
    if has_b3:
        b3_d = nc.dram_tensor("b3b", [P, D], f32, kind="ExternalInput").ap()

    with tile.TileContext(nc) as tc:
        with (
            tc.tile_pool(name="const", bufs=1) as const_pool,
            # keep every x tile resident (8 x 512 KiB = 4 MiB of 24 MiB
            # SBUF) so loads stream back-to-back instead of waiting for
            # each tile's full compute chain to release its slot
            tc.tile_pool(name="xin", bufs=N_TILES) as xin,
            tc.tile_pool(name="xtp", bufs=4) as xtp,
            tc.tile_pool(name="outp", bufs=4) as outp,
            tc.tile_pool(name="small", bufs=4) as small,
            tc.tile_pool(name="ptp", bufs=6, space="PSUM") as ptp,
            tc.tile_pool(name="pd", bufs=2, space="PSUM") as pd,
        ):
            ident = const_pool.tile([P, P], f32, tag="ident")
            make_identity(nc, ident[:])
            # W^T chunks: wt_sb[p, c, l] = W[l, c*128+p]
            wt_sb = const_pool.tile([P, N_CH, L], f32, tag="wt")
            nc.sync.dma_start(wt_sb[:],
                              wt_d.rearrange("(c p) l -> p c l", p=P))
            if has_b3:
                b3 = const_pool.tile([P, D], f32, tag="b3")
                nc.sync.dma_start(b3[:], b3_d[:])

            for t in range(N_TILES):
                xt = xin.tile([P, D], f32, tag="xt")
                nc.sync.dma_start(xt[:], x_d[t * P:(t + 1) * P, :])

                # transpose the tile: xts[p, c*128+a] = xt[a, c*128+p]
                xts = xtp.tile([P, D], f32, tag="xts")
                for g in range(N_CH // 4):
                    tp = ptp.tile([P, 512], f32, tag="tp")
                    for j in range(4):
                        c = 4 * g + j
                        nc.tensor.transpose(
                            tp[:, j * P:(j + 1) * P],
                            xt[:, c * P:(c + 1) * P],
                            ident[:])
                    nc.vector.tensor_copy(xts[:, g * 512:(g + 1) * 512],
                                          tp[:])

                # d[b, l] = sum_d x0[b, d] W[l, d] via PE, accumulated
                # over the 8 d-chunks
                dps = pd.tile([P, L], f32, tag="dps")
                for c in range(N_CH):
                    nc.tensor.matmul(
                        dps[:],
                        xts[:, c * P:(c + 1) * P],
                        wt_sb[:, c, :],
                        start=(c == 0),
                        stop=(c == N_CH - 1))

                # sigma recurrence: sigma_{l+1} = sigma_l*(1+d_l) + beta_l
                sig = small.tile([P, 1], f32, tag="sig1")
                nc.vector.tensor_scalar_add(sig[:], dps[:, 0:1],
                                            1.0 + betas[0])
                for l in (1, 2):
                    nsig = small.tile([P, 1], f32, tag=f"sig{l + 1}")
                    nc.vector.scalar_tensor_tensor(
                        out=nsig[:], in0=dps[:, l:l + 1], scalar=1.0,
                        in1=sig[:], op0=add, op1=mult)
                    if betas[l] != 0.0:
                        nc.vector.tensor_scalar_add(nsig[:], nsig[:],
                                                    float(betas[l]))
                    sig = nsig

                # out = x0 * sigma_3 (+ B3) — scaled copy on ACT
                ot = outp.tile([P, D], f32, tag="ot")
                nc.scalar.mul(ot[:], xt[:], sig[:])
                if has_b3:
                    nc.vector.tensor_add(ot[:], ot[:], b3[:])
                nc.sync.dma_start(out_d[t * P:(t + 1) * P, :], ot[:])

    nc.compile()
    return nc


def predict_time_ns(trace_path=None):
    """Single-core timeline-sim of the kernel program (cost-model time in
    ns).  SPMD data-parallel with no collectives, so per-core time ==
    kernel time.  Optionally writes a perfetto trace."""
    from trails.perfetto import LazyPerfetto
    for _m in ("enable_explicit_ordering", "reserve_process_order",
               "add_counter", "add_flow", "add_instant"):
        if not hasattr(LazyPerfetto, _m):
            setattr(LazyPerfetto, _m, lambda self, *a, **k: None)
    from concourse.timeline_sim import TimelineSim

    nc = _build_program([0.0, 0.0, 0.0], False)
    tlsim = TimelineSim(nc, trace=trace_path is not None)
    tlsim.simulate()
    if trace_path is not None and tlsim.perfetto is not None:
        tlsim.perfetto.save(trace_path)
    return tlsim.time


def kernel(x, W, b):
    global LAST_RESULTS
    from concourse.bass_utils import run_bass_kernel_spmd

    x = np.ascontiguousarray(np.asarray(x, dtype=np.float32))
    W = np.asarray(W, dtype=np.float32)
    b = np.asarray(b, dtype=np.float32)

    # Host precompute: beta_l = (sum_{j<l} b_j) . w_l  and B_3 = sum_l b_l.
    Bl = np.zeros(D, dtype=np.float64)
    betas = []
    for l in range(L):
        betas.append(float(Bl @ W[l].astype(np.float64)))
        Bl = Bl + b[l].astype(np.float64)
    B3 = Bl.astype(np.float32)
    has_b3 = bool(np.any(B3))

    nc = _build_program(betas, has_b3)

    wt_host = np.ascontiguousarray(W.T)  # [D, L]
    in_maps = []
    for i in range(N_CORES):
        m = {"x": x[i * B_SH:(i + 1) * B_SH], "wt": wt_host}
        if has_b3:
            m["b3b"] = np.ascontiguousarray(np.broadcast_to(B3, (P, D)))
        in_maps.append(m)

    res = run_bass_kernel_spmd(nc, in_maps, core_ids=list(range(N_CORES)))
    LAST_RESULTS = res
    out = np.concatenate([res.results[i]["out"] for i in range(N_CORES)],
                         axis=0)
    return out
